# revision 23
# baseline (speedup 1.0000x reference)
"""Trainium2 Bass kernel for nn_MultiAttention (two-head Bahdanau sparsemax
attention + gated merge).

Strategy: data-parallel over batch across 8 NeuronCores. The host stages two
views of each values tensor: a transposed fp32 copy [K, S] (cast to fp32r by
the DMA) that feeds the proj_keys matmuls with contraction-on-partitions, and
a bf16 natural copy [S, K] that feeds the (tiny-support) context matmul.
Scores pipeline: psum[h,s] = Wk.T @ valuesT; ACT fuses tanh + per-partition
query bias; PE reduces v.T @ tanh via M=1 matmuls. Sparsemax is solved with
Newton iterations on f(tau) = sum relu(z - tau) - 1 (exact for piecewise
linear f, converges from tau0 = max(z) - 1 in < 10 steps).
"""
import sys
import numpy as np

for _p in ("/opt/trn_rl_repo", "/root/.axon_site", "/root/.axon_site/_ro/trn_rl_repo",
           "/root/.axon_site/_ro/pypackages"):
    if _p not in sys.path:
        sys.path.append(_p)

import ml_dtypes

# bass_utils' axon trace path hard-imports antenv.axon_hooks; ship a stub so a
# stray BASS_TRACE=1 degrades to "no trace" instead of crashing.
try:
    import antenv.axon_hooks  # noqa: F401
except Exception:
    import types as _types
    try:
        import antenv as _antenv
        _m = _types.ModuleType("antenv.axon_hooks")
        _m.get_axon_ntff_profile_hook = lambda: None
        _m.set_axon_ntff_profile_hook = lambda h: None
        sys.modules["antenv.axon_hooks"] = _m
        _antenv.axon_hooks = _m
    except Exception:
        pass

import concourse.bass as bass
import concourse.mybir as mybir
import concourse.tile as tile
from concourse.bass_utils import run_bass_kernel_spmd

dt = mybir.dt
AF = mybir.ActivationFunctionType
OP = mybir.AluOpType
AX = mybir.AxisListType

B, S, K, H = 32, 4096, 512, 512
NCORES = 8
BPC = B // NCORES          # batches per core
SCH = 32                   # s-chunks per problem; chunk c holds rows s = p*32+c
ST = S // 512              # 8 s-tiles of 512 for the proj_keys matmul
NEG = -1e9
NEWTON_ITERS = 8

VT_DT = dt.float32r        # dtype of the proj_keys matmul operands


def _fix_sync_waits(nc, max_waits=1):
    """TRN2 engine instruction words hold only one sync-wait. After Tile
    scheduling, move each excess wait onto its own inserted NoOp."""
    n_fixed = 0
    for fn in nc.m.functions:
        for bb in fn.blocks:
            new_insts = []
            for inst in bb.instructions:
                si = getattr(inst, "sync_info", None)
                if si is not None and si.on_wait and len(si.on_wait) > max_waits:
                    waits = list(si.on_wait)
                    for w in waits[max_waits:]:
                        d = mybir.InstNoOp(name=f"waitfix_{nc.next_id()}",
                                           ins=[], outs=[], bass_nofuse=True,
                                           text_hint="waitfix")
                        d.engine = inst.engine
                        d.sync_info = mybir.SyncInfo(on_wait=[w], on_update=[])
                        new_insts.append(d)
                    inst.sync_info = mybir.SyncInfo(on_wait=waits[:max_waits],
                                                    on_update=list(si.on_update))
                    n_fixed += 1
                new_insts.append(inst)
            bb.instructions[:] = new_insts
    return n_fixed


def _build():
    nc = bass.Bass("TRN2", target_bir_lowering=False, debug=False,
                   num_devices=NCORES)

    def din(name, shape, dtp=dt.float32):
        return nc.dram_tensor(name, shape, dtp, kind="ExternalInput").ap()

    qT_in = din("queryT", [H, BPC])                       # host-transposed
    valsT_in = {"a": din("valsT_a", [BPC, K, S]), "b": din("valsT_b", [BPC, K, S])}
    valsN_in = {"a": din("valsN_a", [BPC, S, K], dt.bfloat16),
                "b": din("valsN_b", [BPC, S, K], dt.bfloat16)}
    mask_in = {"a": din("mask_a", [BPC, 1, S], dt.uint8),
               "b": din("mask_b", [BPC, 1, S], dt.uint8)}
    Wk_in = {"a": din("Wk_a", [K, H]), "b": din("Wk_b", [K, H])}
    Wq_in = {"a": din("Wq_a", [H, H]), "b": din("Wq_b", [H, H])}
    vT_in = {"a": din("vT_a", [128, 4]), "b": din("vT_b", [128, 4])}
    Wg_in = din("Wg", [H + 2 * K, 2])
    bg_in = din("bg", [2, 1])
    Wu_in = {"a": din("Wu_a", [K + H, H]), "b": din("Wu_b", [K + H, H])}
    bu_in = {"a": din("bu_a", [1, H]), "b": din("bu_b", [1, H])}

    att_out = nc.dram_tensor("att", [BPC, 1, H], dt.float32, kind="ExternalOutput").ap()
    al_out = {"a": nc.dram_tensor("al_a", [BPC, 1, S], dt.float32, kind="ExternalOutput").ap(),
              "b": nc.dram_tensor("al_b", [BPC, 1, S], dt.float32, kind="ExternalOutput").ap()}

    with tile.TileContext(nc) as tc:
        _emit(nc, tc, qT_in, valsT_in, valsN_in, mask_in, Wk_in, Wq_in, vT_in,
              Wg_in, bg_in, Wu_in, bu_in, att_out, al_out)
    _fix_sync_waits(nc)
    return nc


def _emit(nc, tc, qT_in, valsT_in, valsN_in, mask_in, Wk_in, Wq_in, vT_in,
          Wg_in, bg_in, Wu_in, bu_in, att_out, al_out):
    import contextlib
    ctx = contextlib.ExitStack()
    with ctx:
        sb = ctx.enter_context(tc.tile_pool(name="sb", bufs=1))
        valsT = ctx.enter_context(tc.tile_pool(name="valsT", bufs=3))
        valsN = ctx.enter_context(tc.tile_pool(name="valsN", bufs=72))
        work = ctx.enter_context(tc.tile_pool(name="work", bufs=4))
        nwt = ctx.enter_context(tc.tile_pool(name="nwt", bufs=3))
        ps_mm = ctx.enter_context(tc.tile_pool(name="ps_mm", bufs=4, space="PSUM"))
        ps_z = ctx.enter_context(tc.tile_pool(name="ps_z", bufs=1, space="PSUM"))
        ps_q = ctx.enter_context(tc.tile_pool(name="ps_q", bufs=1, space="PSUM"))
        ps_sm = ctx.enter_context(tc.tile_pool(name="ps_sm", bufs=1, space="PSUM"))

        V = nc.vector
        A_ = nc.scalar
        T_ = nc.tensor

        # ---------------- weights / constants (cast during DMA) ----------------
        def load_cast(name, ap_dram, shape, dtp, rearr=None):
            t = sb.tile(shape, dtp, tag=name)
            nc.gpsimd.dma_start(out=t[:], in_=ap_dram if rearr is None else rearr)
            return t

        def load(name, ap_dram, shape, dtp=dt.float32, rearr=None):
            t0 = sb.tile(shape, dtp, tag=name)
            nc.sync.dma_start(t0[:], ap_dram if rearr is None else rearr)
            return t0

        # Wk/Wq as lhsT chunks [k=128p, kc, hc, 128h] in VT_DT
        Wk = {h: load_cast("Wk" + h, Wk_in[h], [128, 4, 4, 128], VT_DT,
                           rearr=Wk_in[h].rearrange("(kc p) (hc m) -> p kc hc m", p=128, m=128))
              for h in "ab"}
        Wq = {h: load("Wq" + h, Wq_in[h], [128, 4, 4, 128],
                      rearr=Wq_in[h].rearrange("(kc p) (hc m) -> p kc hc m", p=128, m=128))
              for h in "ab"}
        Wu = {h: load_cast("Wu" + h, Wu_in[h], [128, 8, H], dt.bfloat16,
                           rearr=Wu_in[h].rearrange("(c p) m -> p c m", p=128))
              for h in "ab"}
        Wg = load_cast("Wg", Wg_in, [128, 12, 2], dt.bfloat16,
                       rearr=Wg_in.rearrange("(c p) m -> p c m", p=128))

        qT_r = load("qT_r", qT_in, [128, 4, BPC],
                    rearr=qT_in.rearrange("(c p) b -> p c b", p=128))
        qT_b = load_cast("qT_b", qT_in, [128, 4, BPC], dt.bfloat16,
                         rearr=qT_in.rearrange("(c p) b -> p c b", p=128))
        vT = {h: load_cast("vT" + h, vT_in[h], [128, 4], VT_DT) for h in "ab"}

        bu = {h: load("bu0" + h, bu_in[h], [1, H]) for h in "ab"}
        bg0 = load("bg0", bg_in, [2, 1])

        ones = sb.tile([128, 128], dt.float32, tag="ones")
        V.memset(ones[:], 1.0)

        m8 = {}
        for h in "ab":
            m8[h] = load("m8" + h, mask_in[h], [128, BPC, SCH],
                         rearr=mask_in[h].rearrange("b one (p c) -> p b c", c=SCH),
                         dtp=dt.uint8)

        ident = sb.tile([128, 128], dt.float32, tag="ident")
        V.memset(ident[:], 0.0)
        nc.gpsimd.affine_select(out=ident[:], in_=ident[:],
                                compare_op=OP.not_equal, fill=1.0, base=0,
                                pattern=[[-1, 128]], channel_multiplier=1)

        ctxT = sb.tile([128, 2, BPC, 4], dt.bfloat16, tag="ctxT")
        cands = sb.tile([2, H], dt.float32, tag="cands")

        # ---------------- per-problem pipeline ----------------
        for b in range(BPC):
            for hi, h in enumerate("ab"):
                # proj_qT columns: psum [128, 4hc]; 16 small MMs
                pq = ps_sm.tile([128, 4], dt.float32, tag="sm")
                for hc in range(4):
                    for kc in range(4):
                        T_.matmul(pq[:, hc:hc + 1], Wq[h][:, kc, hc, :],
                                  qT_r[:, kc, b:b + 1],
                                  start=(kc == 0), stop=(kc == 3))
                qbias = work.tile([128, 4], dt.float32, tag="qbias")
                V.tensor_copy(qbias[:], pq[:])

                # natural bf16 values tiles for ctx (prefetch; strided rows s=p*32+c)
                ntiles = []
                for c in range(SCH):
                    vt = valsN.tile([128, K], dt.bfloat16, tag="vt")
                    nc.sync.dma_start(vt[:], valsN_in[h][b, c::SCH, :])
                    ntiles.append(vt)

                # scores land in zz [128, 32] (s = p*32 + c)
                zz = nwt.tile([128, SCH], dt.float32, tag="zz")
                for st in range(ST):
                    # transposed values tile [k=128p, kc, 512s] cast to f32r
                    tvt = valsT.tile([128, 4, 512], VT_DT, tag="tvt")
                    nc.gpsimd.dma_start(
                        out=tvt[:],
                        in_=valsT_in[h][b, :, st * 512:(st + 1) * 512]
                        .rearrange("(kc p) s -> p kc s", p=128))
                    pz = ps_z.tile([1, 512], dt.float32, tag="pz")
                    for hc in range(4):
                        pk = ps_mm.tile([128, 512], dt.float32, tag="pk")
                        for kc in range(4):
                            T_.matmul(pk[:], Wk[h][:, kc, hc, :], tvt[:, kc, :],
                                      start=(kc == 0), stop=(kc == 3))
                        tt = work.tile([128, 512], VT_DT, tag="tt")
                        A_.activation(tt[:], pk[:], AF.Tanh,
                                      bias=qbias[:, hc:hc + 1], scale=1.0)
                        T_.matmul(pz[:], vT[h][:, hc:hc + 1], tt[:],
                                  start=(hc == 0), stop=(hc == 3),
                                  skip_group_check=True)
                    zrow = work.tile([1, 512], dt.float32, tag="zrow")
                    A_.copy(zrow[:], pz[:])
                    nc.sync.dma_start(zz[st * 16:(st + 1) * 16, :], zrow[:])

                # mask
                zm = nwt.tile([128, SCH], dt.float32, tag="zm")
                V.memset(zm[:], NEG)
                V.copy_predicated(zm[:], m8[h][:, b, :], zz[:])
                # tau0 = max(z) - 1
                zred = nwt.tile([128, 1], dt.float32, tag="zred")
                V.tensor_reduce(zred[:], zm[:], axis=AX.X, op=OP.max)
                pzt = ps_sm.tile([1, 128], dt.float32, tag="sm")
                T_.transpose(pzt[:], zred[:], ident[:])
                tau_row = nwt.tile([1, 1], dt.float32, tag="tau_row")
                V.tensor_reduce(tau_row[:], pzt[:], axis=AX.X, op=OP.max)
                V.tensor_scalar(tau_row[:], tau_row[:], 1.0, None, OP.subtract)
                taub = ps_sm.tile([128, 1], dt.float32, tag="taub")
                T_.matmul(taub[:], ones[0:1, :], tau_row[:], start=True, stop=True)
                taus = nwt.tile([128, 1], dt.float32, tag="taus")
                V.tensor_copy(taus[:], taub[:])
                rs = nwt.tile([128, 2], dt.float32, tag="rs")
                relu_d = nwt.tile([128, SCH], dt.float32, tag="relu_d")
                dmb = nwt.tile([128, 1], dt.float32, tag="dmb")
                for it in range(NEWTON_ITERS):
                    # relu(z - tau) and its per-partition sum
                    V.tensor_scalar(relu_d[:], zm[:], taus[:, 0:1], 0.0,
                                    OP.subtract, OP.max)
                    V.tensor_reduce(rs[:, 0:1], relu_d[:], axis=AX.X, op=OP.add)
                    # count(z > tau) per partition
                    V.tensor_scalar(dmb[:].broadcast_to([128, SCH]), zm[:],
                                    taus[:, 0:1], 0.0, OP.is_gt, OP.add,
                                    accum_out=rs[:, 1:2])
                    pseg = ps_sm.tile([1, 2], dt.float32, tag="sm")
                    T_.matmul(pseg[:], ones[:, 0:1], rs[:], start=True, stop=True)
                    sm1 = nwt.tile([1, 2], dt.float32, tag="sm1")
                    V.tensor_scalar(sm1[:, 0:1], pseg[:, 0:1], 1.0, None, OP.subtract)
                    V.reciprocal(sm1[:, 1:2], pseg[:, 1:2])
                    delta = nwt.tile([1, 1], dt.float32, tag="delta")
                    V.tensor_tensor(delta[:], sm1[:, 0:1], sm1[:, 1:2], op=OP.mult)
                    V.tensor_tensor(tau_row[:], tau_row[:], delta[:], op=OP.add)
                    taub = ps_sm.tile([128, 1], dt.float32, tag="taub")
                    T_.matmul(taub[:], ones[0:1, :], tau_row[:], start=True, stop=True)
                    taus = nwt.tile([128, 1], dt.float32, tag="taus")
                    V.tensor_copy(taus[:], taub[:])
                alpha = nwt.tile([128, SCH], dt.float32, tag="alpha")
                V.tensor_scalar(alpha[:], zm[:], taus[:, 0:1], 0.0,
                                OP.subtract, OP.max)
                nc.sync.dma_start(
                    al_out[h][b, 0, :].rearrange("(p c) -> p c", c=SCH), alpha[:])
                alpha_mm = nwt.tile([128, SCH], dt.bfloat16, tag="alpha_mm")
                V.tensor_copy(alpha_mm[:], alpha[:])

                # context: [1, K] = sum_c alpha_col_c.T @ values_tile_c
                pctx = ps_q.tile([1, K], dt.float32, tag="pctx")
                for c in range(SCH):
                    T_.matmul(pctx[:], alpha_mm[:, c:c + 1], ntiles[c][:],
                              start=(c == 0), stop=(c == SCH - 1))
                ctx_row = work.tile([1, K], dt.float32, tag="ctx_row")
                A_.copy(ctx_row[:], pctx[:])
                for cc in range(4):
                    pt = ps_sm.tile([128, 1], dt.float32, tag="sm")
                    T_.transpose(pt[:], ctx_row[:, cc * 128:(cc + 1) * 128],
                                 ones[0:1, 0:1])
                    V.tensor_copy(ctxT[:, hi, b, cc:cc + 1], pt[:])

        # ---------------- tail: gate + candidates + merge ----------------
        for b in range(BPC):
            pgl = ps_sm.tile([2, 1], dt.float32, tag="sm")
            for kc in range(12):
                if kc < 4:
                    col = qT_b[:, kc, b:b + 1]
                elif kc < 8:
                    col = ctxT[:, 0, b, kc - 4:kc - 3]
                else:
                    col = ctxT[:, 1, b, kc - 8:kc - 7]
                T_.matmul(pgl[:], Wg[:, kc, :], col, start=(kc == 0), stop=(kc == 11))
            gl = work.tile([2, 1], dt.float32, tag="gl")
            V.tensor_tensor(gl[:], pgl[:], bg0[:], op=OP.add)
            ge = work.tile([2, 1], dt.float32, tag="ge")
            A_.activation(ge[:], gl[:], AF.Exp)
            psum2 = ps_sm.tile([1, 1], dt.float32, tag="sm")
            T_.matmul(psum2[:], ones[0:2, 0:1], ge[:], start=True, stop=True)
            sinv = work.tile([1, 1], dt.float32, tag="sinv")
            V.reciprocal(sinv[:], psum2[:])
            psb = ps_sm.tile([2, 1], dt.float32, tag="sm")
            T_.matmul(psb[:], ones[0:1, 0:2], sinv[:], start=True, stop=True)
            gw = work.tile([2, 1], dt.float32, tag="gw")
            V.tensor_tensor(gw[:], ge[:], psb[:], op=OP.mult)
            gw_mm = work.tile([2, 1], dt.bfloat16, tag="gw_mm")
            V.tensor_copy(gw_mm[:], gw[:])

            for hi, h in enumerate("ab"):
                pc = ps_q.tile([1, H], dt.float32, tag="pctx")
                for kc in range(8):
                    col = qT_b[:, kc, b:b + 1] if kc < 4 else ctxT[:, hi, b, kc - 4:kc - 3]
                    T_.matmul(pc[:], col, Wu[h][:, kc, :], start=(kc == 0), stop=(kc == 7))
                xg = work.tile([1, H], dt.float32, tag="xg")
                V.tensor_tensor(xg[:], pc[:], bu[h][:], op=OP.add)
                candrow = work.tile([1, H], dt.float32, tag="candrow")
                A_.activation(candrow[:], xg[:], AF.Tanh)
                nc.sync.dma_start(cands[hi:hi + 1, :], candrow[:])
            cands_mm = work.tile([2, H], dt.bfloat16, tag="cands_mm")
            V.tensor_copy(cands_mm[:], cands[:])
            patt = ps_q.tile([1, H], dt.float32, tag="pctx")
            T_.matmul(patt[:], gw_mm[:], cands_mm[:], start=True, stop=True)
            att_row = work.tile([1, H], dt.float32, tag="att_row")
            V.tensor_copy(att_row[:], patt[:])
            nc.sync.dma_start(att_out[b, 0, :].unsqueeze(0), att_row[:])


_NC_CACHE = None
_LAST_RES = None


def kernel(**inputs):
    global _NC_CACHE
    if _NC_CACHE is None:
        _NC_CACHE = _build()
    nc = _NC_CACHE

    bf16 = ml_dtypes.bfloat16
    q = np.asarray(inputs["query"], np.float32)          # [B,1,H]
    qT_full = np.ascontiguousarray(q.reshape(B, H).T)    # [H, B]
    va = np.asarray(inputs["values_a"], np.float32)
    vb = np.asarray(inputs["values_b"], np.float32)
    vaT = np.ascontiguousarray(va.transpose(0, 2, 1))    # [B, K, S]
    vbT = np.ascontiguousarray(vb.transpose(0, 2, 1))
    vaN = np.ascontiguousarray(va.astype(bf16))
    vbN = np.ascontiguousarray(vb.astype(bf16))
    vTa = np.ascontiguousarray(
        np.asarray(inputs["v_a"], np.float32).reshape(4, 128).T)  # [128,4]
    vTb = np.ascontiguousarray(
        np.asarray(inputs["v_b"], np.float32).reshape(4, 128).T)

    in_maps = []
    for core in range(NCORES):
        sl = slice(core * BPC, (core + 1) * BPC)
        in_maps.append({
            "queryT": np.ascontiguousarray(qT_full[:, sl]),
            "valsT_a": vaT[sl], "valsT_b": vbT[sl],
            "valsN_a": vaN[sl], "valsN_b": vbN[sl],
            "mask_a": np.ascontiguousarray(np.asarray(inputs["mask_a"]).view(np.uint8)[sl]),
            "mask_b": np.ascontiguousarray(np.asarray(inputs["mask_b"]).view(np.uint8)[sl]),
            "Wk_a": np.asarray(inputs["Wk_a"], np.float32),
            "Wk_b": np.asarray(inputs["Wk_b"], np.float32),
            "Wq_a": np.asarray(inputs["Wq_a"], np.float32),
            "Wq_b": np.asarray(inputs["Wq_b"], np.float32),
            "vT_a": vTa, "vT_b": vTb,
            "Wg": np.asarray(inputs["Wg"], np.float32),
            "bg": np.asarray(inputs["bg"], np.float32).reshape(2, 1),
            "Wu_a": np.asarray(inputs["Wu_a"], np.float32),
            "Wu_b": np.asarray(inputs["Wu_b"], np.float32),
            "bu_a": np.asarray(inputs["bu_a"], np.float32).reshape(1, H),
            "bu_b": np.asarray(inputs["bu_b"], np.float32).reshape(1, H),
        })

    res = run_bass_kernel_spmd(nc, in_maps, list(range(NCORES)))
    global _LAST_RES
    _LAST_RES = res
    att = np.concatenate([r["att"] for r in res.results], axis=0)
    al_a = np.concatenate([r["al_a"] for r in res.results], axis=0)
    al_b = np.concatenate([r["al_b"] for r in res.results], axis=0)
    return att, al_a, al_b


# revision 24
# speedup vs baseline: 1.1662x; 1.1662x over previous
"""Trainium2 Bass kernel for nn_MultiAttention (two-head Bahdanau sparsemax
attention + gated merge).

Strategy: data-parallel over batch across 8 NeuronCores. The host stages two
views of each values tensor: a transposed fp32 copy [K, S] (cast to fp32r by
the DMA) that feeds the proj_keys matmuls with contraction-on-partitions, and
a bf16 natural copy [S, K] that feeds the (tiny-support) context matmul.
Scores pipeline: psum[h,s] = Wk.T @ valuesT; ACT fuses tanh + per-partition
query bias; PE reduces v.T @ tanh via M=1 matmuls. Sparsemax is solved with
Newton iterations on f(tau) = sum relu(z - tau) - 1 (exact for piecewise
linear f, converges from tau0 = max(z) - 1 in < 10 steps).
"""
import sys
import numpy as np

for _p in ("/opt/trn_rl_repo", "/root/.axon_site", "/root/.axon_site/_ro/trn_rl_repo",
           "/root/.axon_site/_ro/pypackages"):
    if _p not in sys.path:
        sys.path.append(_p)

import ml_dtypes

# bass_utils' axon trace path hard-imports antenv.axon_hooks; ship a stub so a
# stray BASS_TRACE=1 degrades to "no trace" instead of crashing.
try:
    import antenv.axon_hooks  # noqa: F401
except Exception:
    import types as _types
    try:
        import antenv as _antenv
        _m = _types.ModuleType("antenv.axon_hooks")
        _m.get_axon_ntff_profile_hook = lambda: None
        _m.set_axon_ntff_profile_hook = lambda h: None
        sys.modules["antenv.axon_hooks"] = _m
        _antenv.axon_hooks = _m
    except Exception:
        pass

import concourse.bass as bass
import concourse.mybir as mybir
import concourse.tile as tile
from concourse.bass_utils import run_bass_kernel_spmd

dt = mybir.dt
AF = mybir.ActivationFunctionType
OP = mybir.AluOpType
AX = mybir.AxisListType

B, S, K, H = 32, 4096, 512, 512
NCORES = 8
BPC = B // NCORES          # batches per core
SCH = 32                   # s-chunks per problem; chunk c holds rows s = p*32+c
ST = S // 512              # 8 s-tiles of 512 for the proj_keys matmul
NEG = -1e9
NEWTON_ITERS = 8

VT_DT = dt.float32r        # dtype of the proj_keys matmul operands


def _fix_sync_waits(nc, max_waits=1):
    """TRN2 engine instruction words hold only one sync-wait. After Tile
    scheduling, move each excess wait onto its own inserted NoOp."""
    n_fixed = 0
    for fn in nc.m.functions:
        for bb in fn.blocks:
            new_insts = []
            for inst in bb.instructions:
                si = getattr(inst, "sync_info", None)
                if si is not None and si.on_wait and len(si.on_wait) > max_waits:
                    waits = list(si.on_wait)
                    for w in waits[max_waits:]:
                        d = mybir.InstNoOp(name=f"waitfix_{nc.next_id()}",
                                           ins=[], outs=[], bass_nofuse=True,
                                           text_hint="waitfix")
                        d.engine = inst.engine
                        d.sync_info = mybir.SyncInfo(on_wait=[w], on_update=[])
                        new_insts.append(d)
                    inst.sync_info = mybir.SyncInfo(on_wait=waits[:max_waits],
                                                    on_update=list(si.on_update))
                    n_fixed += 1
                new_insts.append(inst)
            bb.instructions[:] = new_insts
    return n_fixed


def _build():
    nc = bass.Bass("TRN2", target_bir_lowering=False, debug=False,
                   num_devices=NCORES)

    def din(name, shape, dtp=dt.float32):
        return nc.dram_tensor(name, shape, dtp, kind="ExternalInput").ap()

    qT_in = din("queryT", [H, BPC])                       # host-transposed
    valsT_in = {"a": din("valsT_a", [BPC, K, S]), "b": din("valsT_b", [BPC, K, S])}
    valsN_in = {"a": din("valsN_a", [BPC, S, K], dt.bfloat16),
                "b": din("valsN_b", [BPC, S, K], dt.bfloat16)}
    mask_in = {"a": din("mask_a", [BPC, 1, S], dt.uint8),
               "b": din("mask_b", [BPC, 1, S], dt.uint8)}
    Wk_in = {"a": din("Wk_a", [K, H]), "b": din("Wk_b", [K, H])}
    Wq_in = {"a": din("Wq_a", [H, H]), "b": din("Wq_b", [H, H])}
    vT_in = {"a": din("vT_a", [128, 4]), "b": din("vT_b", [128, 4])}
    Wg_in = din("Wg", [H + 2 * K, 2])
    bg_in = din("bg", [2, 1])
    Wu_in = {"a": din("Wu_a", [K + H, H]), "b": din("Wu_b", [K + H, H])}
    bu_in = {"a": din("bu_a", [1, H]), "b": din("bu_b", [1, H])}

    att_out = nc.dram_tensor("att", [BPC, 1, H], dt.float32, kind="ExternalOutput").ap()
    al_out = {"a": nc.dram_tensor("al_a", [BPC, 1, S], dt.float32, kind="ExternalOutput").ap(),
              "b": nc.dram_tensor("al_b", [BPC, 1, S], dt.float32, kind="ExternalOutput").ap()}

    with tile.TileContext(nc) as tc:
        _emit(nc, tc, qT_in, valsT_in, valsN_in, mask_in, Wk_in, Wq_in, vT_in,
              Wg_in, bg_in, Wu_in, bu_in, att_out, al_out)
    _fix_sync_waits(nc)
    return nc


def _emit(nc, tc, qT_in, valsT_in, valsN_in, mask_in, Wk_in, Wq_in, vT_in,
          Wg_in, bg_in, Wu_in, bu_in, att_out, al_out):
    import contextlib
    ctx = contextlib.ExitStack()
    with ctx:
        sb = ctx.enter_context(tc.tile_pool(name="sb", bufs=1))
        valsT = ctx.enter_context(tc.tile_pool(name="valsT", bufs=3))
        valsN = ctx.enter_context(tc.tile_pool(name="valsN", bufs=72))
        work = ctx.enter_context(tc.tile_pool(name="work", bufs=4))
        nwt = ctx.enter_context(tc.tile_pool(name="nwt", bufs=2))
        ps_mm = ctx.enter_context(tc.tile_pool(name="ps_mm", bufs=4, space="PSUM"))
        ps_z = ctx.enter_context(tc.tile_pool(name="ps_z", bufs=1, space="PSUM"))
        ps_q = ctx.enter_context(tc.tile_pool(name="ps_q", bufs=1, space="PSUM"))
        ps_sm = ctx.enter_context(tc.tile_pool(name="ps_sm", bufs=1, space="PSUM"))

        V = nc.vector
        A_ = nc.scalar
        T_ = nc.tensor

        # ---------------- weights / constants (cast during DMA) ----------------
        def load_cast(name, ap_dram, shape, dtp, rearr=None):
            t = sb.tile(shape, dtp, tag=name)
            nc.gpsimd.dma_start(out=t[:], in_=ap_dram if rearr is None else rearr)
            return t

        def load(name, ap_dram, shape, dtp=dt.float32, rearr=None):
            t0 = sb.tile(shape, dtp, tag=name)
            nc.sync.dma_start(t0[:], ap_dram if rearr is None else rearr)
            return t0

        # Wk/Wq as lhsT chunks [k=128p, kc, hc, 128h] in VT_DT
        Wk = {h: load_cast("Wk" + h, Wk_in[h], [128, 4, 4, 128], VT_DT,
                           rearr=Wk_in[h].rearrange("(kc p) (hc m) -> p kc hc m", p=128, m=128))
              for h in "ab"}
        Wq = {h: load("Wq" + h, Wq_in[h], [128, 4, 4, 128],
                      rearr=Wq_in[h].rearrange("(kc p) (hc m) -> p kc hc m", p=128, m=128))
              for h in "ab"}
        Wu = {h: load_cast("Wu" + h, Wu_in[h], [128, 8, H], dt.bfloat16,
                           rearr=Wu_in[h].rearrange("(c p) m -> p c m", p=128))
              for h in "ab"}
        Wg = load_cast("Wg", Wg_in, [128, 12, 2], dt.bfloat16,
                       rearr=Wg_in.rearrange("(c p) m -> p c m", p=128))

        qT_r = load("qT_r", qT_in, [128, 4, BPC],
                    rearr=qT_in.rearrange("(c p) b -> p c b", p=128))
        qT_b = load_cast("qT_b", qT_in, [128, 4, BPC], dt.bfloat16,
                         rearr=qT_in.rearrange("(c p) b -> p c b", p=128))
        vT = {h: load_cast("vT" + h, vT_in[h], [128, 4], VT_DT) for h in "ab"}

        bu = {h: load("bu0" + h, bu_in[h], [1, H]) for h in "ab"}
        bg0 = load("bg0", bg_in, [2, 1])

        ones = sb.tile([128, 128], dt.float32, tag="ones")
        V.memset(ones[:], 1.0)

        m8 = {}
        for h in "ab":
            m8[h] = load("m8" + h, mask_in[h], [128, BPC, SCH],
                         rearr=mask_in[h].rearrange("b one (p c) -> p b c", c=SCH),
                         dtp=dt.uint8)

        ident = sb.tile([128, 128], dt.float32, tag="ident")
        V.memset(ident[:], 0.0)
        nc.gpsimd.affine_select(out=ident[:], in_=ident[:],
                                compare_op=OP.not_equal, fill=1.0, base=0,
                                pattern=[[-1, 128]], channel_multiplier=1)

        ctxT = sb.tile([128, 2, BPC, 4], dt.bfloat16, tag="ctxT")
        cands = sb.tile([2, H], dt.float32, tag="cands")

        # ---------------- per-problem pipeline ----------------
        for b in range(BPC):
            for hi, h in enumerate("ab"):
                # proj_qT columns: psum [128, 4hc]; 16 small MMs
                pq = ps_sm.tile([128, 4], dt.float32, tag="sm")
                for hc in range(4):
                    for kc in range(4):
                        T_.matmul(pq[:, hc:hc + 1], Wq[h][:, kc, hc, :],
                                  qT_r[:, kc, b:b + 1],
                                  start=(kc == 0), stop=(kc == 3))
                qbias = work.tile([128, 4], dt.float32, tag="qbias")
                V.tensor_copy(qbias[:], pq[:])

                # natural bf16 values tiles for ctx (prefetch; strided rows s=p*32+c)
                ntiles = []
                for c in range(SCH):
                    vt = valsN.tile([128, K], dt.bfloat16, tag="vt")
                    nc.sync.dma_start(vt[:], valsN_in[h][b, c::SCH, :])
                    ntiles.append(vt)

                # scores land in zz [128, 32] (s = p*32 + c)
                zz = nwt.tile([128, SCH], dt.float32, tag="zz")
                for st in range(ST):
                    # transposed values tile [k=128p, kc, 512s] cast to f32r
                    tvt = valsT.tile([128, 4, 512], VT_DT, tag="tvt")
                    nc.gpsimd.dma_start(
                        out=tvt[:],
                        in_=valsT_in[h][b, :, st * 512:(st + 1) * 512]
                        .rearrange("(kc p) s -> p kc s", p=128))
                    pz = ps_z.tile([1, 512], dt.float32, tag="pz")
                    for hc in range(4):
                        pk = ps_mm.tile([128, 512], dt.float32, tag="pk")
                        for kc in range(4):
                            T_.matmul(pk[:], Wk[h][:, kc, hc, :], tvt[:, kc, :],
                                      start=(kc == 0), stop=(kc == 3))
                        tt = work.tile([128, 512], VT_DT, tag="tt")
                        A_.activation(tt[:], pk[:], AF.Tanh,
                                      bias=qbias[:, hc:hc + 1], scale=1.0)
                        T_.matmul(pz[:], vT[h][:, hc:hc + 1], tt[:],
                                  start=(hc == 0), stop=(hc == 3),
                                  skip_group_check=True)
                    zrow = work.tile([1, 512], dt.float32, tag="zrow")
                    A_.copy(zrow[:], pz[:])
                    nc.sync.dma_start(zz[st * 16:(st + 1) * 16, :], zrow[:])

                # mask
                zm = nwt.tile([128, SCH], dt.float32, tag="zm")
                V.memset(zm[:], NEG)
                V.copy_predicated(zm[:], m8[h][:, b, :], zz[:])
                # tau0 = max(z) - 1
                zred = nwt.tile([128, 1], dt.float32, tag="zred")
                V.tensor_reduce(zred[:], zm[:], axis=AX.X, op=OP.max)
                pzt = ps_sm.tile([1, 128], dt.float32, tag="sm")
                T_.transpose(pzt[:], zred[:], ident[:])
                tau_row = nwt.tile([1, 1], dt.float32, tag="tau_row")
                V.tensor_reduce(tau_row[:], pzt[:], axis=AX.X, op=OP.max)
                V.tensor_scalar(tau_row[:], tau_row[:], 1.0, None, OP.subtract)
                taub = ps_sm.tile([128, 1], dt.float32, tag="taub")
                T_.matmul(taub[:], ones[0:1, :], tau_row[:], start=True, stop=True)
                taus = nwt.tile([128, 1], dt.float32, tag="taus")
                V.tensor_copy(taus[:], taub[:])
                rs = nwt.tile([128, 2], dt.float32, tag="rs")
                relu_d = nwt.tile([128, SCH], dt.float32, tag="relu_d")
                dmb = nwt.tile([128, 1], dt.float32, tag="dmb")
                for it in range(NEWTON_ITERS):
                    # relu(z - tau) and its per-partition sum
                    V.tensor_scalar(relu_d[:], zm[:], taus[:, 0:1], 0.0,
                                    OP.subtract, OP.max)
                    V.tensor_reduce(rs[:, 0:1], relu_d[:], axis=AX.X, op=OP.add)
                    # count(z > tau) per partition
                    V.tensor_scalar(dmb[:].broadcast_to([128, SCH]), zm[:],
                                    taus[:, 0:1], 0.0, OP.is_gt, OP.add,
                                    accum_out=rs[:, 1:2])
                    pseg = ps_sm.tile([1, 2], dt.float32, tag="sm")
                    T_.matmul(pseg[:], ones[:, 0:1], rs[:], start=True, stop=True)
                    sm1 = nwt.tile([1, 2], dt.float32, tag="sm1")
                    V.tensor_scalar(sm1[:, 0:1], pseg[:, 0:1], 1.0, None, OP.subtract)
                    V.reciprocal(sm1[:, 1:2], pseg[:, 1:2])
                    delta = nwt.tile([1, 1], dt.float32, tag="delta")
                    V.tensor_tensor(delta[:], sm1[:, 0:1], sm1[:, 1:2], op=OP.mult)
                    V.tensor_tensor(tau_row[:], tau_row[:], delta[:], op=OP.add)
                    taub = ps_sm.tile([128, 1], dt.float32, tag="taub")
                    T_.matmul(taub[:], ones[0:1, :], tau_row[:], start=True, stop=True)
                    taus = nwt.tile([128, 1], dt.float32, tag="taus")
                    V.tensor_copy(taus[:], taub[:])
                alpha = nwt.tile([128, SCH], dt.float32, tag="alpha")
                V.tensor_scalar(alpha[:], zm[:], taus[:, 0:1], 0.0,
                                OP.subtract, OP.max)
                nc.sync.dma_start(
                    al_out[h][b, 0, :].rearrange("(p c) -> p c", c=SCH), alpha[:])
                alpha_mm = nwt.tile([128, SCH], dt.bfloat16, tag="alpha_mm")
                V.tensor_copy(alpha_mm[:], alpha[:])

                # context: [1, K] = sum_c alpha_col_c.T @ values_tile_c
                pctx = ps_q.tile([1, K], dt.float32, tag="pctx")
                for c in range(SCH):
                    T_.matmul(pctx[:], alpha_mm[:, c:c + 1], ntiles[c][:],
                              start=(c == 0), stop=(c == SCH - 1))
                ctx_row = work.tile([1, K], dt.float32, tag="ctx_row")
                A_.copy(ctx_row[:], pctx[:])
                for cc in range(4):
                    pt = ps_sm.tile([128, 1], dt.float32, tag="sm")
                    T_.transpose(pt[:], ctx_row[:, cc * 128:(cc + 1) * 128],
                                 ones[0:1, 0:1])
                    V.tensor_copy(ctxT[:, hi, b, cc:cc + 1], pt[:])

        # ---------------- tail: gate + candidates + merge ----------------
        for b in range(BPC):
            pgl = ps_sm.tile([2, 1], dt.float32, tag="sm")
            for kc in range(12):
                if kc < 4:
                    col = qT_b[:, kc, b:b + 1]
                elif kc < 8:
                    col = ctxT[:, 0, b, kc - 4:kc - 3]
                else:
                    col = ctxT[:, 1, b, kc - 8:kc - 7]
                T_.matmul(pgl[:], Wg[:, kc, :], col, start=(kc == 0), stop=(kc == 11))
            gl = work.tile([2, 1], dt.float32, tag="gl")
            V.tensor_tensor(gl[:], pgl[:], bg0[:], op=OP.add)
            ge = work.tile([2, 1], dt.float32, tag="ge")
            A_.activation(ge[:], gl[:], AF.Exp)
            psum2 = ps_sm.tile([1, 1], dt.float32, tag="sm")
            T_.matmul(psum2[:], ones[0:2, 0:1], ge[:], start=True, stop=True)
            sinv = work.tile([1, 1], dt.float32, tag="sinv")
            V.reciprocal(sinv[:], psum2[:])
            psb = ps_sm.tile([2, 1], dt.float32, tag="sm")
            T_.matmul(psb[:], ones[0:1, 0:2], sinv[:], start=True, stop=True)
            gw = work.tile([2, 1], dt.float32, tag="gw")
            V.tensor_tensor(gw[:], ge[:], psb[:], op=OP.mult)
            gw_mm = work.tile([2, 1], dt.bfloat16, tag="gw_mm")
            V.tensor_copy(gw_mm[:], gw[:])

            for hi, h in enumerate("ab"):
                pc = ps_q.tile([1, H], dt.float32, tag="pctx")
                for kc in range(8):
                    col = qT_b[:, kc, b:b + 1] if kc < 4 else ctxT[:, hi, b, kc - 4:kc - 3]
                    T_.matmul(pc[:], col, Wu[h][:, kc, :], start=(kc == 0), stop=(kc == 7))
                xg = work.tile([1, H], dt.float32, tag="xg")
                V.tensor_tensor(xg[:], pc[:], bu[h][:], op=OP.add)
                candrow = work.tile([1, H], dt.float32, tag="candrow")
                A_.activation(candrow[:], xg[:], AF.Tanh)
                nc.sync.dma_start(cands[hi:hi + 1, :], candrow[:])
            cands_mm = work.tile([2, H], dt.bfloat16, tag="cands_mm")
            V.tensor_copy(cands_mm[:], cands[:])
            patt = ps_q.tile([1, H], dt.float32, tag="pctx")
            T_.matmul(patt[:], gw_mm[:], cands_mm[:], start=True, stop=True)
            att_row = work.tile([1, H], dt.float32, tag="att_row")
            V.tensor_copy(att_row[:], patt[:])
            nc.sync.dma_start(att_out[b, 0, :].unsqueeze(0), att_row[:])


_NC_CACHE = None
_LAST_RES = None


def kernel(**inputs):
    global _NC_CACHE
    if _NC_CACHE is None:
        _NC_CACHE = _build()
    nc = _NC_CACHE

    bf16 = ml_dtypes.bfloat16
    q = np.asarray(inputs["query"], np.float32)          # [B,1,H]
    qT_full = np.ascontiguousarray(q.reshape(B, H).T)    # [H, B]
    va = np.asarray(inputs["values_a"], np.float32)
    vb = np.asarray(inputs["values_b"], np.float32)
    vaT = np.ascontiguousarray(va.transpose(0, 2, 1))    # [B, K, S]
    vbT = np.ascontiguousarray(vb.transpose(0, 2, 1))
    vaN = np.ascontiguousarray(va.astype(bf16))
    vbN = np.ascontiguousarray(vb.astype(bf16))
    vTa = np.ascontiguousarray(
        np.asarray(inputs["v_a"], np.float32).reshape(4, 128).T)  # [128,4]
    vTb = np.ascontiguousarray(
        np.asarray(inputs["v_b"], np.float32).reshape(4, 128).T)

    in_maps = []
    for core in range(NCORES):
        sl = slice(core * BPC, (core + 1) * BPC)
        in_maps.append({
            "queryT": np.ascontiguousarray(qT_full[:, sl]),
            "valsT_a": vaT[sl], "valsT_b": vbT[sl],
            "valsN_a": vaN[sl], "valsN_b": vbN[sl],
            "mask_a": np.ascontiguousarray(np.asarray(inputs["mask_a"]).view(np.uint8)[sl]),
            "mask_b": np.ascontiguousarray(np.asarray(inputs["mask_b"]).view(np.uint8)[sl]),
            "Wk_a": np.asarray(inputs["Wk_a"], np.float32),
            "Wk_b": np.asarray(inputs["Wk_b"], np.float32),
            "Wq_a": np.asarray(inputs["Wq_a"], np.float32),
            "Wq_b": np.asarray(inputs["Wq_b"], np.float32),
            "vT_a": vTa, "vT_b": vTb,
            "Wg": np.asarray(inputs["Wg"], np.float32),
            "bg": np.asarray(inputs["bg"], np.float32).reshape(2, 1),
            "Wu_a": np.asarray(inputs["Wu_a"], np.float32),
            "Wu_b": np.asarray(inputs["Wu_b"], np.float32),
            "bu_a": np.asarray(inputs["bu_a"], np.float32).reshape(1, H),
            "bu_b": np.asarray(inputs["bu_b"], np.float32).reshape(1, H),
        })

    res = run_bass_kernel_spmd(nc, in_maps, list(range(NCORES)))
    global _LAST_RES
    _LAST_RES = res
    att = np.concatenate([r["att"] for r in res.results], axis=0)
    al_a = np.concatenate([r["al_a"] for r in res.results], axis=0)
    al_b = np.concatenate([r["al_b"] for r in res.results], axis=0)
    return att, al_a, al_b
